# revision 38
# baseline (speedup 1.0000x reference)
"""BNN MLP (784 -> 2048 -> 2048 -> 2048 -> 10, sign activations) on 8 TRN2 cores.

Strategy:
  - Data-parallel: batch 16384 sharded 2048/core; weights replicated.
  - fc1 (real-valued x @ sign(W1).T): x split into fp16 hi+lo (captures fp32
    to ~2^-23); both passes concatenated along K (784+784 = 1568 rows) and run
    as 12 full matmul chunks + one 32-row partial chunk per chain. The four
    partial chunks of a 4-chain group run CONCURRENTLY in the PE's four 32-row
    sub-array groups via tile_position (saves 3/4 of the 13th chunk's
    512-cycle streams; 12.25 effective chunks = the packing floor).
  - BatchNorm + hardtanh + sign folds into a per-feature threshold computed
    on host in fp64: sign(bn(h)) == sign(h - t). Layer outputs are written as
    {0,1} in fp8e4 (exact) by a single DVE is_ge per PSUM tile.
  - fc2/fc3: {0,1} inputs and +-1 weights in fp8e4 => products and fp32 PSUM
    accumulation are exact integers. DoubleRow perf mode, weight-stationary
    over all batch groups. The {0,1} representation is corrected
    algebraically via row-sum constants folded into the next threshold.
  - fc4 + log_softmax on device: DoubleRow matmuls (w4 zero-padded 10->16
    cols to satisfy the dual-fp8 LDWEIGHTS ISA rule), feature-major logits
    with an exact bias add (bit-identical to the reference), PE transpose to
    batch-major, batched log_softmax via broadcast APs, ACT Exp/Ln.
  - w1 is +-1 so it is stored fp8 (mixed fp8-stationary x fp16-moving
    matmul, products exact); w1/w2/w3 live in separate pools so each rep's
    weight DMA prefetches during the previous rep's later phases.
  - fc1 full-chunk matmuls run in DoublePixel perf mode (2 fp16 moving
    pixels/cycle; HW-measured 169 vs 332 ns per N=512 MM, bit-identical).
  - Thresholding is split across DVE and the otherwise-idle ACT engine:
    fc1: even fg4 groups use one batched DVE tensor_tensor is_ge ({0,1}
    encoding) per 4-chain group; odd fg4 groups use ACT Sign(p-u) (+-1
    encoding, sign-exact, ties->0 matching the reference). fc2 weight rows
    for {0,1}-encoded features are doubled and the row-subset rowsum folds
    into u2. fc2/fc3: half the f-tiles threshold on ACT via
    sigmoid(2^18*(p-u)), which provably rounds to exact {0,1} in fp8
    because p is an exact integer; features whose threshold sits too close
    to an integer are host-detected and kept on DVE.
  - fc4+log_softmax batched: all 4 fc4 outputs land in one 4-bank PSUM
    slot (transposes in a second), logits = 2*raw4 + (b4 - rowsum4) in one
    fused ACT pass, one DVE/ACT instr per softmax stage across all 16
    row-tiles.
  - is_fmap_onezero is OFF: HW-measured it slows DR matmuls ~9% (the
    earlier claim that it helps did not reproduce; PE streams 1 col/cycle
    regardless, so DoubleRow's 2-rows-per-column is the only packing win).
"""

import sys

sys.path.insert(0, "/opt/trn_rl_repo")

from contextlib import ExitStack

import ml_dtypes
import numpy as np

import concourse.bass as bass
import concourse.mybir as mybir
import concourse.tile as tile
from concourse import bacc
from concourse.bass_utils import run_bass_kernel_spmd
from concourse.masks import make_identity

BN_EPS = 1e-5
N_CORES = 8
B, D_IN, H, C = 16384, 784, 2048, 10
BS = B // N_CORES            # 2048 batch rows per core
NB = 512                     # moving free dim per matmul
NBG = BS // NB               # 4 batch groups per core
KC1 = 13                     # fc1 k-chunks: 2*784=1568 padded to 13*128=1664
K1P = KC1 * 128
FH = H // 128                # 16 feature tiles
DP2 = H // 256               # 8 DoubleRow k-pair chunks for fc2/fc3
F8 = mybir.dt.float8e4
F16 = mybir.dt.float16
F32 = mybir.dt.float32

_CACHE = {}


ACT_SCALE = float(2 ** 18)


ACT1_FG4 = (1, 3)  # fg4 groups whose fc1 thresholds run on ACT (Sign, +-1)


def _build_program(do_compile=True, reps=1, phases=4, big_psum=True, swi=True, colsplit=False, oz=False, fc4_dr=True, fc4v2=True, dp=True, act1=True, act2_mask=None, act3_mask=None, abl=()):
    if act2_mask is None:
        act2_mask = _CACHE.get("act2", ())
    if act3_mask is None:
        act3_mask = _CACHE.get("act3", ())
    nc = bacc.Bacc("TRN2", target_bir_lowering=False, debug=False,
                   num_devices=N_CORES)

    # bg-major / f-major layouts so each DMA slice is fully contiguous
    # (long descriptor lines) and compute can start after the first slice.
    xcat = nc.dram_tensor("xcat", [NBG, 128, KC1, NB], F16,
                          kind="ExternalInput").ap()
    w1 = nc.dram_tensor("w1", [FH, 128, KC1, 128], F8,
                        kind="ExternalInput").ap()
    if swi:
        w2 = nc.dram_tensor("w2", [DP2, 128, FH, 256], F8,
                            kind="ExternalInput").ap()
        w3 = nc.dram_tensor("w3", [DP2, 128, FH, 256], F8,
                            kind="ExternalInput").ap()
    else:
        w2 = nc.dram_tensor("w2", [DP2, 128, 2, H], F8,
                            kind="ExternalInput").ap()
        w3 = nc.dram_tensor("w3", [DP2, 128, 2, H], F8,
                            kind="ExternalInput").ap()
    w4 = nc.dram_tensor("w4", [DP2, 128, 2, 16], F8, kind="ExternalInput").ap()
    # per-feature decision thresholds (fp64 host math, {0,1} algebra)
    u1 = nc.dram_tensor("u1", [128, FH], F32, kind="ExternalInput").ap()
    u2 = nc.dram_tensor("u2", [128, FH], F32, kind="ExternalInput").ap()
    u3 = nc.dram_tensor("u3", [128, FH], F32, kind="ExternalInput").ap()
    # -ACT_SCALE * u2/u3, bias terms for the ACT-sigmoid threshold path
    u2s = nc.dram_tensor("u2s", [128, FH], F32, kind="ExternalInput").ap()
    u3s = nc.dram_tensor("u3s", [128, FH], F32, kind="ExternalInput").ap()
    u1n = nc.dram_tensor("u1n", [128, FH], F32, kind="ExternalInput").ap()
    rs4h = nc.dram_tensor("rs4h", [C, 1], F32, kind="ExternalInput").ap()
    b4 = nc.dram_tensor("b4", [C, 1], F32, kind="ExternalInput").ap()
    c4 = nc.dram_tensor("c4", [C, 1], F32, kind="ExternalInput").ap()
    out = nc.dram_tensor("out", [BS, C], F32, kind="ExternalOutput").ap()

    def bcast(ap_2d, n):
        # [128, k] -> [128, k, n] with a zero-step inner dim (free replication)
        return bass.AP(ap_2d.tensor, ap_2d.offset, ap_2d.ap + [[0, n]])

    with tile.TileContext(nc) as tc, ExitStack() as ctx:
        const = ctx.enter_context(tc.tile_pool(name="const", bufs=1))
        pwa = ctx.enter_context(tc.tile_pool(name="pwa", bufs=1))
        pwb = ctx.enter_context(tc.tile_pool(name="pwb", bufs=1))
        pwc = ctx.enter_context(tc.tile_pool(name="pwc", bufs=1))
        px = ctx.enter_context(tc.tile_pool(name="px", bufs=3))
        psa = ctx.enter_context(tc.tile_pool(name="psa", bufs=1))
        psb = ctx.enter_context(tc.tile_pool(name="psb", bufs=1))
        small = ctx.enter_context(tc.tile_pool(name="small", bufs=2))
        # big_psum: slot = [128, BS] f32 (4 banks) x 2 bufs = 8 banks
        # else: slot = [128, NB] (1 bank) x 8 bufs
        pmm = ctx.enter_context(tc.tile_pool(
            name="pmm", bufs=2 if big_psum else 8, space="PSUM"))

        ident = const.tile([128, 128], F32)
        make_identity(nc, ident[:])
        u1_sb = const.tile([128, FH], F32)
        u2_sb = const.tile([128, FH], F32)
        u3_sb = const.tile([128, FH], F32)
        u2s_sb = const.tile([128, FH], F32)
        u3s_sb = const.tile([128, FH], F32)
        u1n_sb = const.tile([128, FH], F32)
        nc.sync.dma_start(u2s_sb[:], u2s[:])
        nc.sync.dma_start(u3s_sb[:], u3s[:])
        nc.sync.dma_start(u1n_sb[:], u1n[:])
        rs4h_sb = const.tile([C, 1], F32)
        b4_sb = const.tile([C, 1], F32)
        c4_sb = const.tile([C, 1], F32)
        nc.sync.dma_start(c4_sb[:], c4[:])
        w4_sb = const.tile([128, DP2, 2, 16], F8)
        nc.sync.dma_start(u1_sb[:], u1[:])
        nc.sync.dma_start(u2_sb[:], u2[:])
        nc.sync.dma_start(u3_sb[:], u3[:])
        nc.sync.dma_start(rs4h_sb[:], rs4h[:])
        nc.sync.dma_start(b4_sb[:], b4[:])
        nc.sync.dma_start(w4_sb[:], w4.rearrange("d p j m -> p d j m"))

        _ABL_X = {}
        _ABL_W = {}
        for rep in range(reps):
            if "static_w1" in abl and rep > 0:
                w1_sb = _ABL_W["w1"]
            else:
                w1_sb = pwa.tile([128, FH, KC1, 128], F8, tag="w1")
                for f in range(FH):
                    nc.sync.dma_start(w1_sb[:, f, :, :], w1[f])
                _ABL_W["w1"] = w1_sb
            if swi:
                w2_sb = pwb.tile([128, DP2, FH, 256], F8, tag="wb")
                nc.sync.dma_start(w2_sb[:], w2.rearrange("d p f m -> p d f m"))
            else:
                w2_sb = pwb.tile([128, DP2, 2, H], F8, tag="wb")
                nc.sync.dma_start(w2_sb[:], w2.rearrange("d p j m -> p d j m"))
            if swi:
                w3_sb = pwc.tile([128, DP2, FH, 256], F8, tag="wc")
                nc.sync.dma_start(w3_sb[:], w3.rearrange("d p f m -> p d f m"))
            else:
                w3_sb = pwc.tile([128, DP2, 2, H], F8, tag="wc")
                nc.sync.dma_start(w3_sb[:], w3.rearrange("d p j m -> p d j m"))

            s1 = psa.tile([128, NBG, FH, NB], F8, tag="sa")  # {0,1} outputs
            s2 = psb.tile([128, NBG, FH, NB], F8, tag="sb")

            # ---- fc1: raw1 = xcat.T @ w1cat; s1 = (raw1 >= u1) ----
            xts = []
            for bg in range(NBG):
                bsl = bass.ts(bg, NB)
                if "static_x" in abl:
                    if rep == 0:
                        xt = const.tile([128, KC1, NB], F16, tag=f"x{bg}")
                        nc.sync.dma_start(xt[:], xcat[bg])
                        xts.append(xt)
                        _ABL_X[bg] = xt
                    else:
                        xt = _ABL_X[bg]
                else:
                    # x loads ride the ACT HWDGE ring: the SP ring's weight
                    # loads (w1/w2/w3 ~10MB/rep, semaphore-gated) would
                    # head-of-line block them at every rep boundary
                    xt = px.tile([128, KC1, NB], F16, tag="x")
                    nc.scalar.dma_start(xt[:], xcat[bg])
                for fg4 in range(FH // 4):
                    # 4 chains share one 4-bank tile; their 32-row final
                    # chunks run concurrently in the 4 PE row groups.
                    # Full chunks stream 2 fp16 pixels/cycle (DoublePixel:
                    # HW-measured 169 vs 332 ns/MM at N=512, exact).
                    pm1 = mybir.MatmulPerfMode.DoublePixel if dp else None
                    p = pmm.tile([128, 4, NB], F32, tag="mm",
                                 name=f"p1_{rep}_{bg}_{fg4}")
                    kcs = range(6) if "kc7" in abl else range(KC1 - 1)
                    for kc in kcs:
                        for j in range(4):
                            f = 4 * fg4 + j
                            nc.tensor.matmul(p[:, j, :], w1_sb[:, f, kc, :],
                                             xt[:, kc, :], start=(kc == 0),
                                             stop=False, perf_mode=pm1)
                    for j in range(4):
                        f = 4 * fg4 + j
                        nc.tensor.matmul(
                            p[:, j, :],
                            w1_sb[32 * j:32 * j + 32, f, KC1 - 1, :],
                            xt[32 * j:32 * j + 32, KC1 - 1, :],
                            start=False, stop=True, tile_position=(32 * j, 0))
                    if "dve8" in abl:
                        for j in range(4):
                            f = 4 * fg4 + j
                            nc.vector.tensor_scalar(s1[:, bg, f, :64],
                                                    p[:, j, :64],
                                                    u1_sb[:, f:f + 1], None,
                                                    mybir.AluOpType.is_ge)
                    elif act1 and fg4 in ACT1_FG4:
                        # ACT-engine threshold: sign(p - u) in {-1,0,+1};
                        # fp32 subtract is sign-exact, so the decision is
                        # identical to is_ge (ties -> 0, matching reference).
                        # These tiles are +-1-encoded; w2 rows compensate.
                        # j=0 stays on DVE (62/38 element split ~ matches
                        # ACT's slower per-element rate).
                        nc.vector.tensor_scalar(
                            s1[:, bg, 4 * fg4, :], p[:, 0, :],
                            u1_sb[:, 4 * fg4:4 * fg4 + 1], None,
                            mybir.AluOpType.is_ge)
                        for j in range(1, 4):
                            f = 4 * fg4 + j
                            nc.scalar.activation(
                                s1[:, bg, f, :], p[:, j, :],
                                mybir.ActivationFunctionType.Sign,
                                bias=u1n_sb[:, f:f + 1])
                    elif "no_dve1" not in abl:
                        # one DVE instr per 4-chain group: thresholds held in
                        # [128,4] slice broadcast along the batch dim
                        nc.vector.tensor_tensor(
                            out=s1[:, bg, 4 * fg4:4 * fg4 + 4, :],
                            in0=p[:],
                            in1=bcast(u1_sb[:, 4 * fg4:4 * fg4 + 4], NB),
                            op=mybir.AluOpType.is_ge)

            if phases < 2:
                continue
            # ---- fc2: weight-stationary DoubleRow over a 4-bank PSUM tile;
            # one is_ge over all BS columns per feature tile ----
            for f in range(FH):
                fsl = bass.ts(f, 128)
                if big_psum:
                    p = pmm.tile([128, BS], F32, tag="mm", name=f"p2_{f}")
                    pslices = [p[:, bass.ts(bg, NB)] for bg in range(NBG)]
                else:
                    ps = [pmm.tile([128, NB], F32, tag="mm", name=f"p2_{f}_{i}")
                          for i in range(NBG)]
                    pslices = [t[:] for t in ps]
                pm = (mybir.MatmulPerfMode.DoubleRowSwInterleave if swi
                      else mybir.MatmulPerfMode.DoubleRow)
                dmax = DP2 // 2 if "fc2half" in abl else DP2
                for d in range(dmax):
                    if colsplit:
                        # alternate 64-col halves of the PE array so the
                        # weight load of one half overlaps the other's stream
                        for h in range(2):
                            wl = w2_sb[:, d, :, f * 128 + 64 * h:
                                       f * 128 + 64 * (h + 1)]
                            for bg in range(NBG):
                                sl = pslices[bg]
                                nc.tensor.matmul(
                                    sl[64 * h:64 * (h + 1), :], wl,
                                    s1[:, bg, 2 * d:2 * d + 2, :],
                                    start=(d == 0), stop=(d == dmax - 1),
                                    perf_mode=pm, skip_group_check=True)
                        continue
                    w2l = (w2_sb[:, d, f, :] if swi
                           else w2_sb[:, d, :, fsl])
                    for bg in range(NBG):
                        mm = nc.tensor.matmul(
                            pslices[bg], w2l,
                            s1[:, bg, 2 * d:2 * d + 2, :],
                            start=(d == 0), stop=(d == dmax - 1),
                            perf_mode=pm)
                        if oz:
                            mm.is_fmap_onezero = True
                if big_psum and f in act2_mask:
                    # ACT-engine threshold: sigmoid(S*(p-u)) rounds to exact
                    # {0,1} in fp8 (host certified |p-u|*S >= ~25 per feature)
                    nc.scalar.activation(
                        s2[:, :, f, :],
                        p[:].rearrange("p (g n) -> p g n", n=NB),
                        mybir.ActivationFunctionType.Sigmoid,
                        bias=u2s_sb[:, f:f + 1], scale=ACT_SCALE)
                elif big_psum:
                    nc.vector.tensor_scalar(
                        s2[:, :, f, :],
                        p[:].rearrange("p (g n) -> p g n", n=NB),
                        u2_sb[:, f:f + 1], None, mybir.AluOpType.is_ge)
                else:
                    for bg in range(NBG):
                        nc.vector.tensor_scalar(
                            s2[:, bg, f, :], pslices[bg],
                            u2_sb[:, f:f + 1], None, mybir.AluOpType.is_ge)

            if phases < 3:
                continue
            # s1 slot reused for layer-3 output
            s3 = psa.tile([128, NBG, FH, NB], F8, tag="sa")

            # ---- fc3: same structure as fc2 ----
            for f in range(FH):
                fsl = bass.ts(f, 128)
                if big_psum:
                    p = pmm.tile([128, BS], F32, tag="mm", name=f"p3_{f}")
                    pslices = [p[:, bass.ts(bg, NB)] for bg in range(NBG)]
                else:
                    ps = [pmm.tile([128, NB], F32, tag="mm", name=f"p3_{f}_{i}")
                          for i in range(NBG)]
                    pslices = [t[:] for t in ps]
                pm = (mybir.MatmulPerfMode.DoubleRowSwInterleave if swi
                      else mybir.MatmulPerfMode.DoubleRow)
                for d in range(dmax):
                    if colsplit:
                        for h in range(2):
                            wl = w3_sb[:, d, :, f * 128 + 64 * h:
                                       f * 128 + 64 * (h + 1)]
                            for bg in range(NBG):
                                sl = pslices[bg]
                                nc.tensor.matmul(
                                    sl[64 * h:64 * (h + 1), :], wl,
                                    s2[:, bg, 2 * d:2 * d + 2, :],
                                    start=(d == 0), stop=(d == dmax - 1),
                                    perf_mode=pm, skip_group_check=True)
                        continue
                    w3l = (w3_sb[:, d, f, :] if swi
                           else w3_sb[:, d, :, fsl])
                    for bg in range(NBG):
                        mm = nc.tensor.matmul(
                            pslices[bg], w3l,
                            s2[:, bg, 2 * d:2 * d + 2, :],
                            start=(d == 0), stop=(d == dmax - 1),
                            perf_mode=pm)
                        if oz:
                            mm.is_fmap_onezero = True
                if big_psum and f in act3_mask:
                    nc.scalar.activation(
                        s3[:, :, f, :],
                        p[:].rearrange("p (g n) -> p g n", n=NB),
                        mybir.ActivationFunctionType.Sigmoid,
                        bias=u3s_sb[:, f:f + 1], scale=ACT_SCALE)
                elif big_psum:
                    nc.vector.tensor_scalar(
                        s3[:, :, f, :],
                        p[:].rearrange("p (g n) -> p g n", n=NB),
                        u3_sb[:, f:f + 1], None, mybir.AluOpType.is_ge)
                else:
                    for bg in range(NBG):
                        nc.vector.tensor_scalar(
                            s3[:, bg, f, :], pslices[bg],
                            u3_sb[:, f:f + 1], None, mybir.AluOpType.is_ge)

            if phases < 4:
                continue
            if fc4v2:
                # ---- fc4 + log_softmax, batched across all 4 bgs ----
                # all 4 [16,512] fc4 outputs live in ONE 4-bank pmm slot
                # (bank bg each); all 16 transposes land in a second slot.
                # One DVE/ACT op per tail stage over [.., 4*NB] instead of
                # 4x per-bg round trips (fewer PE<->DVE stalls).
                p4q = pmm.tile([128, NBG, NB], F32, tag="mm", name="p4q")
                for bg in range(NBG):
                    for d in range(DP2):
                        nc.tensor.matmul(
                            p4q[:16, bg, :], w4_sb[:, d, :, :],
                            s3[:, bg, 2 * d:2 * d + 2, :],
                            start=(d == 0), stop=(d == DP2 - 1),
                            perf_mode=mybir.MatmulPerfMode.DoubleRow)
                # logits = 2*raw4 + (b4 - rowsum4): one fused ACT pass straight
                # from PSUM (c4 rounded once to fp32; ~1e-4 logit error, far
                # inside the 2e-2 gate, and nothing thresholds downstream)
                lg = small.tile([C, NBG, NB], F32, tag="lg", bufs=1)
                nc.scalar.activation(lg[:], p4q[:C, :, :],
                                     mybir.ActivationFunctionType.Identity,
                                     bias=c4_sb[:], scale=2.0)
                ptq = pmm.tile([128, NBG, NB], F32, tag="mm", name="ptq")
                for bg in range(NBG):
                    for t in range(NB // 128):
                        nc.tensor.transpose(
                            ptq[:, bg, t * C:(t + 1) * C],
                            lg[:, bg, bass.ts(t, 128)], ident[:C, :C])
                TB = NB // 128  # 4 transpose tiles per bg
                lt = ptq[:, :, :TB * C].rearrange("p g (t c) -> p g t c",
                                                  c=C)
                mxn = small.tile([128, NBG, TB], F32, tag="mxn")
                nc.vector.tensor_reduce(out=mxn[:], in_=lt,
                                        op=mybir.AluOpType.max,
                                        axis=mybir.AxisListType.X,
                                        negate=True)
                sh = small.tile([128, NBG, TB, C], F32, tag="sh")
                nc.vector.tensor_tensor(out=sh[:], in0=lt,
                                        in1=bcast(mxn[:], C),
                                        op=mybir.AluOpType.add)
                ex = small.tile([128, NBG, TB, C], F32, tag="ex")
                nc.scalar.activation(ex[:], sh[:],
                                     mybir.ActivationFunctionType.Exp)
                sm = small.tile([128, NBG, TB], F32, tag="sm")
                nc.vector.tensor_reduce(out=sm[:], in_=ex[:],
                                        op=mybir.AluOpType.add,
                                        axis=mybir.AxisListType.X)
                lsm = small.tile([128, NBG, TB], F32, tag="lsm")
                nc.scalar.activation(lsm[:], sm[:],
                                     mybir.ActivationFunctionType.Ln)
                ot = small.tile([128, NBG, TB, C], F32, tag="ot")
                nc.vector.tensor_tensor(out=ot[:], in0=sh[:],
                                        in1=bcast(lsm[:], C),
                                        op=mybir.AluOpType.subtract)
                for bg in range(NBG):
                    nc.sync.dma_start(
                        out[bg * NB:(bg + 1) * NB, :].rearrange(
                            "(t p) c -> p t c", p=128),
                        ot[:, bg, :, :])
                continue
            # ---- fc4 + log_softmax (per-bg tail) ----
            for bg in range(NBG):
                bsl = bass.ts(bg, NB)
                p4full = pmm.tile([128, NB], F32, tag="mm", name=f"p4_{bg}")
                p4 = p4full[:C, :]
                if fc4_dr:
                    for d in range(DP2):
                        nc.tensor.matmul(p4full[:16, :], w4_sb[:, d, :, :],
                                         s3[:, bg, 2 * d:2 * d + 2, :],
                                         start=(d == 0), stop=(d == DP2 - 1),
                                         perf_mode=mybir.MatmulPerfMode.DoubleRow)
                else:
                    for kc in range(FH):
                        nc.tensor.matmul(p4, w4_sb[:, kc, :], s3[:, bg, kc, :],
                                         start=(kc == 0), stop=(kc == FH - 1))
                # logits = 2*(raw4 - rowsum4/2) + b4: subtract exact (ints),
                # scale+bias rounds once -- bit-identical to (h @ W.T) + b4
                q4 = small.tile([C, NB], F32, tag="q4")
                nc.vector.tensor_scalar(q4[:], p4, rs4h_sb[:], None,
                                        mybir.AluOpType.subtract)
                lg = small.tile([C, NB], F32, tag="lg")
                nc.scalar.activation(lg[:], q4[:],
                                     mybir.ActivationFunctionType.Identity,
                                     bias=b4_sb[:], scale=2.0)
                # transpose 4x [10,128] -> one [128, 4, 10] psum region
                ptf = pmm.tile([128, NB], F32, tag="mm", name=f"pt_{bg}")
                for t in range(NB // 128):
                    nc.tensor.transpose(ptf[:, t * C:(t + 1) * C],
                                        lg[:, bass.ts(t, 128)], ident[:C, :C])
                lt = ptf[:, :4 * C].rearrange("p (t c) -> p t c", c=C)
                # batched log_softmax over the class dim (innermost)
                mxn = small.tile([128, 4], F32, tag="mxn")
                nc.vector.tensor_reduce(out=mxn[:], in_=lt,
                                        op=mybir.AluOpType.max,
                                        axis=mybir.AxisListType.X, negate=True)
                sh = small.tile([128, 4, C], F32, tag="sh")
                nc.vector.tensor_tensor(out=sh[:], in0=lt, in1=bcast(mxn[:], C),
                                        op=mybir.AluOpType.add)
                ex = small.tile([128, 4, C], F32, tag="ex")
                nc.scalar.activation(ex[:], sh[:],
                                     mybir.ActivationFunctionType.Exp)
                sm = small.tile([128, 4], F32, tag="sm")
                nc.vector.tensor_reduce(out=sm[:], in_=ex[:],
                                        op=mybir.AluOpType.add,
                                        axis=mybir.AxisListType.X)
                lsm = small.tile([128, 4], F32, tag="lsm")
                nc.scalar.activation(lsm[:], sm[:],
                                     mybir.ActivationFunctionType.Ln)
                ot = small.tile([128, 4, C], F32, tag="ot")
                nc.vector.tensor_tensor(out=ot[:], in0=sh[:],
                                        in1=bcast(lsm[:], C),
                                        op=mybir.AluOpType.subtract)
                nc.sync.dma_start(
                    out[bg * NB:(bg + 1) * NB, :].rearrange(
                        "(t p) c -> p t c", p=128), ot[:])

    if do_compile:
        nc.compile()
    return nc


def _prep_inputs(inputs, swi=True):
    """Host-side packing: sharding, fp16 split, sign-binarization, fp64
    threshold folding."""
    f64 = np.float64
    x = np.asarray(inputs["x"], np.float32)
    W1s = np.sign(np.asarray(inputs["W1"], np.float32))
    W2s = np.sign(np.asarray(inputs["W2"], np.float32))
    W3s = np.sign(np.asarray(inputs["W3"], np.float32))
    W4s = np.sign(np.asarray(inputs["W4"], np.float32))

    def thr(i):
        g = np.asarray(inputs[f"g{i}"], f64)
        be = np.asarray(inputs[f"be{i}"], f64)
        m = np.asarray(inputs[f"m{i}"], f64)
        v = np.asarray(inputs[f"v{i}"], f64)
        return m - be * np.sqrt(v + BN_EPS) / g

    b1 = np.asarray(inputs["b1"], f64)
    b2 = np.asarray(inputs["b2"], f64)
    b3 = np.asarray(inputs["b3"], f64)
    b4 = np.asarray(inputs["b4"], np.float32)

    # every layer output is {0,1} = (raw >= u); for a {0,1} input layer the
    # true pre-bn activation is h = 2*raw - rowsum + b, so the bn>=0
    # decision h >= t becomes raw >= (rowsum - b + t)/2
    u1 = (thr(1) - b1).astype(np.float32)
    # fc1 output encoding per feature: fg4 groups in ACT1_FG4 emit +-1 via
    # ACT Sign, the rest emit {0,1} via DVE is_ge. For {0,1} rows, double
    # the fc2 weight row (product 2*w*s01 = w*(2*s01)) and shift the
    # threshold by that row-subset's rowsum: h2 = raw2' - rowsum_01 + b2.
    f_idx = np.arange(H) // 128
    s1_is01 = ~np.isin(f_idx // 4, np.asarray(ACT1_FG4)) | (f_idx % 4 == 0)
    W2mod = W2s * np.where(s1_is01[None, :], 2.0, 1.0).astype(np.float32)
    rowsum2_01 = W2s.astype(f64)[:, s1_is01].sum(axis=1)
    u2_64 = rowsum2_01 - b2 + thr(2)
    u2v = u2_64.astype(np.float32)
    rowsum3 = W3s.astype(f64).sum(axis=1)
    u3_64 = (rowsum3 - b3 + thr(3)) / 2.0
    u3 = u3_64.astype(np.float32)

    # ACT-sigmoid threshold path: a feature is eligible iff its threshold is
    # far enough from every integer (matmul outputs are exact ints) that
    # sigmoid(S*(p-u)) provably saturates to {0,1} in fp8, with 4x margin
    # over the fp32 bias-rounding error. Whole 128-feature tiles only.
    def act_mask(u64):
        dz = np.abs(u64 - np.round(u64))
        ok = (dz * ACT_SCALE >= 100.0).reshape(FH, 128).all(axis=1)
        return tuple(f for f in range(FH) if ok[f] and f % 2 == 1)

    _CACHE["act2"] = act_mask(u2_64)
    _CACHE["act3"] = act_mask(u3_64)
    u2s_arr = np.ascontiguousarray(
        (-ACT_SCALE * u2_64).astype(np.float32).reshape(FH, 128).T)
    u3s_arr = np.ascontiguousarray(
        (-ACT_SCALE * u3_64).astype(np.float32).reshape(FH, 128).T)
    # layer 4: logits = 2*(raw4 - rowsum4/2) + b4 = 2*raw4 + c4
    rowsum4 = W4s.astype(f64).sum(axis=1)
    rs4h_v = (rowsum4 / 2.0).astype(np.float32)
    c4_v = (np.asarray(inputs["b4"], f64) - rowsum4).astype(np.float32)

    # fc1 operands: fp16 hi/lo split of x, K = [hi(784) | lo(784) | pad]
    x_hi = x.astype(np.float16)
    x_lo = (x - x_hi.astype(np.float32)).astype(np.float16)
    w1cat = np.zeros((K1P, H), np.float32)
    w1cat[:D_IN] = W1s.T
    w1cat[D_IN:2 * D_IN] = W1s.T
    # [f, p, kc, m]: per-f slice contiguous for long DMA lines
    w1_arr = np.ascontiguousarray(
        w1cat.reshape(KC1, 128, FH, 128).transpose(2, 1, 0, 3)).astype(
        ml_dtypes.float8_e4m3)
    # per-f: move chunk-12 weights into row group f%4 (matches tile_position)
    for f in range(FH):
        j = f % 4
        blk = w1_arr[f, 0:32, KC1 - 1, :].copy()
        w1_arr[f, :, KC1 - 1, :] = 0
        w1_arr[f, 32 * j:32 * j + 32, KC1 - 1, :] = blk

    xcatT = np.zeros((K1P, B), np.float16)
    xcatT[:D_IN] = x_hi.T
    xcatT[D_IN:2 * D_IN] = x_lo.T
    # chunk 12 holds rows 1536:1568 in row-group 0; replicate into groups
    # 1..3 so per-f packed finals can read their own row group
    for i in range(1, 4):
        xcatT[1536 + 32 * i:1536 + 32 * (i + 1)] = xcatT[1536:1568]

    def pack_dr(WsT):
        a = WsT.reshape(DP2, 2, 128, H).transpose(0, 2, 1, 3)
        if not swi:
            return np.ascontiguousarray(a).astype(ml_dtypes.float8_e4m3)
        # SwInterleave: per f-block of 128 cols, pairs (A,B) interleaved
        # with reversed column order: [A127 B127 A126 B126 ... A0 B0]
        a = a.reshape(DP2, 128, 2, FH, 128)          # [d, p, j, f, m]
        a = a[:, :, :, :, ::-1]                      # reverse m
        a = a.transpose(0, 1, 3, 4, 2)               # [d, p, f, m, j]
        return np.ascontiguousarray(
            a.reshape(DP2, 128, FH, 256)).astype(ml_dtypes.float8_e4m3)

    w2_arr = pack_dr(W2mod.T)
    w3_arr = pack_dr(W3s.T)
    w4p = np.zeros((H, 16), np.float32)
    w4p[:, :C] = W4s.T
    w4_arr = np.ascontiguousarray(
        w4p.reshape(DP2, 2, 128, 16).transpose(0, 2, 1, 3)).astype(
        ml_dtypes.float8_e4m3)

    u1_arr = np.ascontiguousarray(u1.reshape(FH, 128).T)
    u1n_arr = np.ascontiguousarray((-u1).reshape(FH, 128).T)
    u2_arr = np.ascontiguousarray(u2v.reshape(FH, 128).T)
    u3_arr = np.ascontiguousarray(u3.reshape(FH, 128).T)
    rs4h_arr = np.ascontiguousarray(rs4h_v.reshape(C, 1))
    b4_arr = np.ascontiguousarray(b4.reshape(C, 1))
    c4_arr = np.ascontiguousarray(c4_v.reshape(C, 1))

    shared = {"w1": w1_arr, "w2": w2_arr, "w3": w3_arr, "w4": w4_arr,
              "u1": u1_arr, "u2": u2_arr, "u3": u3_arr, "rs4h": rs4h_arr,
              "b4": b4_arr, "c4": c4_arr, "u2s": u2s_arr, "u3s": u3s_arr,
              "u1n": u1n_arr}
    in_maps = []
    for c in range(N_CORES):
        # [bg, p, kc, b]: per-bg slice contiguous
        xc = np.ascontiguousarray(
            np.ascontiguousarray(xcatT[:, c * BS:(c + 1) * BS])
            .reshape(KC1, 128, NBG, NB).transpose(2, 1, 0, 3))
        in_maps.append({"xcat": xc, **shared})
    return in_maps


def kernel(**inputs):
    in_maps = _prep_inputs(inputs)
    if "nc" not in _CACHE:
        _CACHE["nc"] = _build_program()
    nc = _CACHE["nc"]
    res = run_bass_kernel_spmd(nc, in_maps, list(range(N_CORES)))
    return np.concatenate([res.results[c]["out"] for c in range(N_CORES)],
                          axis=0).astype(np.float32)



# revision 39
# speedup vs baseline: 1.1061x; 1.1061x over previous
"""BNN MLP (784 -> 2048 -> 2048 -> 2048 -> 10, sign activations) on 8 TRN2 cores.

Strategy:
  - Data-parallel: batch 16384 sharded 2048/core; weights replicated.
  - fc1 (real-valued x @ sign(W1).T): x split into fp16 hi+lo (captures fp32
    to ~2^-23); both passes concatenated along K (784+784 = 1568 rows) and run
    as 12 full matmul chunks + one 32-row partial chunk per chain. The four
    partial chunks of a 4-chain group run CONCURRENTLY in the PE's four 32-row
    sub-array groups via tile_position (saves 3/4 of the 13th chunk's
    512-cycle streams; 12.25 effective chunks = the packing floor).
  - BatchNorm + hardtanh + sign folds into a per-feature threshold computed
    on host in fp64: sign(bn(h)) == sign(h - t). Layer outputs are written as
    {0,1} in fp8e4 (exact) by a single DVE is_ge per PSUM tile.
  - fc2/fc3: {0,1} inputs and +-1 weights in fp8e4 => products and fp32 PSUM
    accumulation are exact integers. DoubleRow perf mode, weight-stationary
    over all batch groups. The {0,1} representation is corrected
    algebraically via row-sum constants folded into the next threshold.
  - fc4 + log_softmax on device: DoubleRow matmuls (w4 zero-padded 10->16
    cols to satisfy the dual-fp8 LDWEIGHTS ISA rule), feature-major logits
    with an exact bias add (bit-identical to the reference), PE transpose to
    batch-major, batched log_softmax via broadcast APs, ACT Exp/Ln.
  - w1 is +-1 so it is stored fp8 (mixed fp8-stationary x fp16-moving
    matmul, products exact); w1/w2/w3 live in separate pools so each rep's
    weight DMA prefetches during the previous rep's later phases.
  - fc1 full-chunk matmuls run in DoublePixel perf mode (2 fp16 moving
    pixels/cycle; HW-measured 169 vs 332 ns per N=512 MM, bit-identical).
  - Thresholding is split across DVE and the otherwise-idle ACT engine:
    fc1: even fg4 groups use one batched DVE tensor_tensor is_ge ({0,1}
    encoding) per 4-chain group; odd fg4 groups use ACT Sign(p-u) (+-1
    encoding, sign-exact, ties->0 matching the reference). fc2 weight rows
    for {0,1}-encoded features are doubled and the row-subset rowsum folds
    into u2. fc2/fc3: half the f-tiles threshold on ACT via
    sigmoid(2^18*(p-u)), which provably rounds to exact {0,1} in fp8
    because p is an exact integer; features whose threshold sits too close
    to an integer are host-detected and kept on DVE.
  - fc4+log_softmax batched: all 4 fc4 outputs land in one 4-bank PSUM
    slot (transposes in a second), logits = 2*raw4 + (b4 - rowsum4) in one
    fused ACT pass, one DVE/ACT instr per softmax stage across all 16
    row-tiles.
  - is_fmap_onezero is OFF: HW-measured it slows DR matmuls ~9% (the
    earlier claim that it helps did not reproduce; PE streams 1 col/cycle
    regardless, so DoubleRow's 2-rows-per-column is the only packing win).
"""

import sys

sys.path.insert(0, "/opt/trn_rl_repo")

from contextlib import ExitStack

import ml_dtypes
import numpy as np

import concourse.bass as bass
import concourse.mybir as mybir
import concourse.tile as tile
from concourse import bacc
from concourse.bass_utils import run_bass_kernel_spmd
from concourse.masks import make_identity

BN_EPS = 1e-5
N_CORES = 8
B, D_IN, H, C = 16384, 784, 2048, 10
BS = B // N_CORES            # 2048 batch rows per core
NB = 512                     # moving free dim per matmul
NBG = BS // NB               # 4 batch groups per core
KC1 = 13                     # fc1 k-chunks: 2*784=1568 padded to 13*128=1664
K1P = KC1 * 128
FH = H // 128                # 16 feature tiles
DP2 = H // 256               # 8 DoubleRow k-pair chunks for fc2/fc3
F8 = mybir.dt.float8e4
F16 = mybir.dt.float16
F32 = mybir.dt.float32

_CACHE = {}


ACT_SCALE = float(2 ** 18)


ACT1_FG4 = (1, 3)  # fg4 groups whose fc1 thresholds run on ACT (Sign, +-1)


def _build_program(do_compile=True, reps=1, phases=4, big_psum=True, swi=True, colsplit=False, oz=False, fc4_dr=True, fc4v2=True, dp=True, act1=True, act2_mask=None, act3_mask=None, abl=()):
    if act2_mask is None:
        act2_mask = _CACHE.get("act2", ())
    if act3_mask is None:
        act3_mask = _CACHE.get("act3", ())
    nc = bacc.Bacc("TRN2", target_bir_lowering=False, debug=False,
                   num_devices=N_CORES)

    # bg-major / f-major layouts so each DMA slice is fully contiguous
    # (long descriptor lines) and compute can start after the first slice.
    xcat = nc.dram_tensor("xcat", [NBG, 128, KC1, NB], F16,
                          kind="ExternalInput").ap()
    w1 = nc.dram_tensor("w1", [FH, 128, KC1, 128], F8,
                        kind="ExternalInput").ap()
    if swi:
        w2 = nc.dram_tensor("w2", [DP2, 128, FH, 256], F8,
                            kind="ExternalInput").ap()
        w3 = nc.dram_tensor("w3", [DP2, 128, FH, 256], F8,
                            kind="ExternalInput").ap()
    else:
        w2 = nc.dram_tensor("w2", [DP2, 128, 2, H], F8,
                            kind="ExternalInput").ap()
        w3 = nc.dram_tensor("w3", [DP2, 128, 2, H], F8,
                            kind="ExternalInput").ap()
    w4 = nc.dram_tensor("w4", [DP2, 128, 2, 16], F8, kind="ExternalInput").ap()
    # per-feature decision thresholds (fp64 host math, {0,1} algebra)
    u1 = nc.dram_tensor("u1", [128, FH], F32, kind="ExternalInput").ap()
    u2 = nc.dram_tensor("u2", [128, FH], F32, kind="ExternalInput").ap()
    u3 = nc.dram_tensor("u3", [128, FH], F32, kind="ExternalInput").ap()
    # -ACT_SCALE * u2/u3, bias terms for the ACT-sigmoid threshold path
    u2s = nc.dram_tensor("u2s", [128, FH], F32, kind="ExternalInput").ap()
    u3s = nc.dram_tensor("u3s", [128, FH], F32, kind="ExternalInput").ap()
    u1n = nc.dram_tensor("u1n", [128, FH], F32, kind="ExternalInput").ap()
    rs4h = nc.dram_tensor("rs4h", [C, 1], F32, kind="ExternalInput").ap()
    b4 = nc.dram_tensor("b4", [C, 1], F32, kind="ExternalInput").ap()
    c4 = nc.dram_tensor("c4", [C, 1], F32, kind="ExternalInput").ap()
    out = nc.dram_tensor("out", [BS, C], F32, kind="ExternalOutput").ap()

    def bcast(ap_2d, n):
        # [128, k] -> [128, k, n] with a zero-step inner dim (free replication)
        return bass.AP(ap_2d.tensor, ap_2d.offset, ap_2d.ap + [[0, n]])

    with tile.TileContext(nc) as tc, ExitStack() as ctx:
        const = ctx.enter_context(tc.tile_pool(name="const", bufs=1))
        pwa = ctx.enter_context(tc.tile_pool(name="pwa", bufs=1))
        pwb = ctx.enter_context(tc.tile_pool(name="pwb", bufs=1))
        pwc = ctx.enter_context(tc.tile_pool(name="pwc", bufs=1))
        px = ctx.enter_context(tc.tile_pool(name="px", bufs=3))
        psa = ctx.enter_context(tc.tile_pool(name="psa", bufs=1))
        psb = ctx.enter_context(tc.tile_pool(name="psb", bufs=1))
        small = ctx.enter_context(tc.tile_pool(name="small", bufs=2))
        # big_psum: slot = [128, BS] f32 (4 banks) x 2 bufs = 8 banks
        # else: slot = [128, NB] (1 bank) x 8 bufs
        pmm = ctx.enter_context(tc.tile_pool(
            name="pmm", bufs=2 if big_psum else 8, space="PSUM"))

        ident = const.tile([128, 128], F32)
        make_identity(nc, ident[:])
        u1_sb = const.tile([128, FH], F32)
        u2_sb = const.tile([128, FH], F32)
        u3_sb = const.tile([128, FH], F32)
        u2s_sb = const.tile([128, FH], F32)
        u3s_sb = const.tile([128, FH], F32)
        u1n_sb = const.tile([128, FH], F32)
        nc.sync.dma_start(u2s_sb[:], u2s[:])
        nc.sync.dma_start(u3s_sb[:], u3s[:])
        nc.sync.dma_start(u1n_sb[:], u1n[:])
        rs4h_sb = const.tile([C, 1], F32)
        b4_sb = const.tile([C, 1], F32)
        c4_sb = const.tile([C, 1], F32)
        nc.sync.dma_start(c4_sb[:], c4[:])
        w4_sb = const.tile([128, DP2, 2, 16], F8)
        nc.sync.dma_start(u1_sb[:], u1[:])
        nc.sync.dma_start(u2_sb[:], u2[:])
        nc.sync.dma_start(u3_sb[:], u3[:])
        nc.sync.dma_start(rs4h_sb[:], rs4h[:])
        nc.sync.dma_start(b4_sb[:], b4[:])
        nc.sync.dma_start(w4_sb[:], w4.rearrange("d p j m -> p d j m"))

        _ABL_X = {}
        _ABL_W = {}
        for rep in range(reps):
            if "static_w1" in abl and rep > 0:
                w1_sb = _ABL_W["w1"]
            else:
                w1_sb = pwa.tile([128, FH, KC1, 128], F8, tag="w1")
                for f in range(FH):
                    nc.sync.dma_start(w1_sb[:, f, :, :], w1[f])
                _ABL_W["w1"] = w1_sb
            if swi:
                w2_sb = pwb.tile([128, DP2, FH, 256], F8, tag="wb")
                nc.sync.dma_start(w2_sb[:], w2.rearrange("d p f m -> p d f m"))
            else:
                w2_sb = pwb.tile([128, DP2, 2, H], F8, tag="wb")
                nc.sync.dma_start(w2_sb[:], w2.rearrange("d p j m -> p d j m"))
            if swi:
                w3_sb = pwc.tile([128, DP2, FH, 256], F8, tag="wc")
                nc.sync.dma_start(w3_sb[:], w3.rearrange("d p f m -> p d f m"))
            else:
                w3_sb = pwc.tile([128, DP2, 2, H], F8, tag="wc")
                nc.sync.dma_start(w3_sb[:], w3.rearrange("d p j m -> p d j m"))

            s1 = psa.tile([128, NBG, FH, NB], F8, tag="sa")  # {0,1} outputs
            s2 = psb.tile([128, NBG, FH, NB], F8, tag="sb")

            # ---- fc1: raw1 = xcat.T @ w1cat; s1 = (raw1 >= u1) ----
            xts = []
            for bg in range(NBG):
                bsl = bass.ts(bg, NB)
                if "static_x" in abl:
                    if rep == 0:
                        xt = const.tile([128, KC1, NB], F16, tag=f"x{bg}")
                        nc.sync.dma_start(xt[:], xcat[bg])
                        xts.append(xt)
                        _ABL_X[bg] = xt
                    else:
                        xt = _ABL_X[bg]
                else:
                    # x loads ride the ACT HWDGE ring: the SP ring's weight
                    # loads (w1/w2/w3 ~10MB/rep, semaphore-gated) would
                    # head-of-line block them at every rep boundary
                    xt = px.tile([128, KC1, NB], F16, tag="x")
                    nc.scalar.dma_start(xt[:], xcat[bg])
                for fg4 in range(FH // 4):
                    # 4 chains share one 4-bank tile; their 32-row final
                    # chunks run concurrently in the 4 PE row groups.
                    # Full chunks stream 2 fp16 pixels/cycle (DoublePixel:
                    # HW-measured 169 vs 332 ns/MM at N=512, exact).
                    pm1 = mybir.MatmulPerfMode.DoublePixel if dp else None
                    p = pmm.tile([128, 4, NB], F32, tag="mm",
                                 name=f"p1_{rep}_{bg}_{fg4}")
                    kcs = range(6) if "kc7" in abl else range(KC1 - 1)
                    for kc in kcs:
                        for j in range(4):
                            f = 4 * fg4 + j
                            nc.tensor.matmul(p[:, j, :], w1_sb[:, f, kc, :],
                                             xt[:, kc, :], start=(kc == 0),
                                             stop=False, perf_mode=pm1)
                    for j in range(4):
                        f = 4 * fg4 + j
                        nc.tensor.matmul(
                            p[:, j, :],
                            w1_sb[32 * j:32 * j + 32, f, KC1 - 1, :],
                            xt[32 * j:32 * j + 32, KC1 - 1, :],
                            start=False, stop=True, tile_position=(32 * j, 0))
                    if "dve8" in abl:
                        for j in range(4):
                            f = 4 * fg4 + j
                            nc.vector.tensor_scalar(s1[:, bg, f, :64],
                                                    p[:, j, :64],
                                                    u1_sb[:, f:f + 1], None,
                                                    mybir.AluOpType.is_ge)
                    elif act1 and fg4 in ACT1_FG4:
                        # ACT-engine threshold: sign(p - u) in {-1,0,+1};
                        # fp32 subtract is sign-exact, so the decision is
                        # identical to is_ge (ties -> 0, matching reference).
                        # These tiles are +-1-encoded; w2 rows compensate.
                        for j in range(4):
                            f = 4 * fg4 + j
                            nc.scalar.activation(
                                s1[:, bg, f, :], p[:, j, :],
                                mybir.ActivationFunctionType.Sign,
                                bias=u1n_sb[:, f:f + 1])
                    elif "no_dve1" not in abl:
                        # one DVE instr per 4-chain group: thresholds held in
                        # [128,4] slice broadcast along the batch dim
                        nc.vector.tensor_tensor(
                            out=s1[:, bg, 4 * fg4:4 * fg4 + 4, :],
                            in0=p[:],
                            in1=bcast(u1_sb[:, 4 * fg4:4 * fg4 + 4], NB),
                            op=mybir.AluOpType.is_ge)

            if phases < 2:
                continue
            # ---- fc2: weight-stationary DoubleRow over a 4-bank PSUM tile;
            # one is_ge over all BS columns per feature tile ----
            for f in range(FH):
                fsl = bass.ts(f, 128)
                if big_psum:
                    p = pmm.tile([128, BS], F32, tag="mm", name=f"p2_{f}")
                    pslices = [p[:, bass.ts(bg, NB)] for bg in range(NBG)]
                else:
                    ps = [pmm.tile([128, NB], F32, tag="mm", name=f"p2_{f}_{i}")
                          for i in range(NBG)]
                    pslices = [t[:] for t in ps]
                pm = (mybir.MatmulPerfMode.DoubleRowSwInterleave if swi
                      else mybir.MatmulPerfMode.DoubleRow)
                dmax = DP2 // 2 if "fc2half" in abl else DP2
                for d in range(dmax):
                    if colsplit:
                        # alternate 64-col halves of the PE array so the
                        # weight load of one half overlaps the other's stream
                        for h in range(2):
                            wl = w2_sb[:, d, :, f * 128 + 64 * h:
                                       f * 128 + 64 * (h + 1)]
                            for bg in range(NBG):
                                sl = pslices[bg]
                                nc.tensor.matmul(
                                    sl[64 * h:64 * (h + 1), :], wl,
                                    s1[:, bg, 2 * d:2 * d + 2, :],
                                    start=(d == 0), stop=(d == dmax - 1),
                                    perf_mode=pm, skip_group_check=True)
                        continue
                    w2l = (w2_sb[:, d, f, :] if swi
                           else w2_sb[:, d, :, fsl])
                    for bg in range(NBG):
                        mm = nc.tensor.matmul(
                            pslices[bg], w2l,
                            s1[:, bg, 2 * d:2 * d + 2, :],
                            start=(d == 0), stop=(d == dmax - 1),
                            perf_mode=pm)
                        if oz:
                            mm.is_fmap_onezero = True
                if big_psum and f in act2_mask:
                    # ACT-engine threshold: sigmoid(S*(p-u)) rounds to exact
                    # {0,1} in fp8 (host certified |p-u|*S >= ~25 per feature)
                    nc.scalar.activation(
                        s2[:, :, f, :],
                        p[:].rearrange("p (g n) -> p g n", n=NB),
                        mybir.ActivationFunctionType.Sigmoid,
                        bias=u2s_sb[:, f:f + 1], scale=ACT_SCALE)
                elif big_psum:
                    nc.vector.tensor_scalar(
                        s2[:, :, f, :],
                        p[:].rearrange("p (g n) -> p g n", n=NB),
                        u2_sb[:, f:f + 1], None, mybir.AluOpType.is_ge)
                else:
                    for bg in range(NBG):
                        nc.vector.tensor_scalar(
                            s2[:, bg, f, :], pslices[bg],
                            u2_sb[:, f:f + 1], None, mybir.AluOpType.is_ge)

            if phases < 3:
                continue
            # s1 slot reused for layer-3 output
            s3 = psa.tile([128, NBG, FH, NB], F8, tag="sa")

            # ---- fc3: same structure as fc2 ----
            for f in range(FH):
                fsl = bass.ts(f, 128)
                if big_psum:
                    p = pmm.tile([128, BS], F32, tag="mm", name=f"p3_{f}")
                    pslices = [p[:, bass.ts(bg, NB)] for bg in range(NBG)]
                else:
                    ps = [pmm.tile([128, NB], F32, tag="mm", name=f"p3_{f}_{i}")
                          for i in range(NBG)]
                    pslices = [t[:] for t in ps]
                pm = (mybir.MatmulPerfMode.DoubleRowSwInterleave if swi
                      else mybir.MatmulPerfMode.DoubleRow)
                for d in range(dmax):
                    if colsplit:
                        for h in range(2):
                            wl = w3_sb[:, d, :, f * 128 + 64 * h:
                                       f * 128 + 64 * (h + 1)]
                            for bg in range(NBG):
                                sl = pslices[bg]
                                nc.tensor.matmul(
                                    sl[64 * h:64 * (h + 1), :], wl,
                                    s2[:, bg, 2 * d:2 * d + 2, :],
                                    start=(d == 0), stop=(d == dmax - 1),
                                    perf_mode=pm, skip_group_check=True)
                        continue
                    w3l = (w3_sb[:, d, f, :] if swi
                           else w3_sb[:, d, :, fsl])
                    for bg in range(NBG):
                        mm = nc.tensor.matmul(
                            pslices[bg], w3l,
                            s2[:, bg, 2 * d:2 * d + 2, :],
                            start=(d == 0), stop=(d == dmax - 1),
                            perf_mode=pm)
                        if oz:
                            mm.is_fmap_onezero = True
                if big_psum and f in act3_mask:
                    nc.scalar.activation(
                        s3[:, :, f, :],
                        p[:].rearrange("p (g n) -> p g n", n=NB),
                        mybir.ActivationFunctionType.Sigmoid,
                        bias=u3s_sb[:, f:f + 1], scale=ACT_SCALE)
                elif big_psum:
                    nc.vector.tensor_scalar(
                        s3[:, :, f, :],
                        p[:].rearrange("p (g n) -> p g n", n=NB),
                        u3_sb[:, f:f + 1], None, mybir.AluOpType.is_ge)
                else:
                    for bg in range(NBG):
                        nc.vector.tensor_scalar(
                            s3[:, bg, f, :], pslices[bg],
                            u3_sb[:, f:f + 1], None, mybir.AluOpType.is_ge)

            if phases < 4:
                continue
            if fc4v2:
                # ---- fc4 + log_softmax, batched across all 4 bgs ----
                # all 4 [16,512] fc4 outputs live in ONE 4-bank pmm slot
                # (bank bg each); all 16 transposes land in a second slot.
                # One DVE/ACT op per tail stage over [.., 4*NB] instead of
                # 4x per-bg round trips (fewer PE<->DVE stalls).
                p4q = pmm.tile([128, NBG, NB], F32, tag="mm", name="p4q")
                for bg in range(NBG):
                    for d in range(DP2):
                        nc.tensor.matmul(
                            p4q[:16, bg, :], w4_sb[:, d, :, :],
                            s3[:, bg, 2 * d:2 * d + 2, :],
                            start=(d == 0), stop=(d == DP2 - 1),
                            perf_mode=mybir.MatmulPerfMode.DoubleRow)
                # logits = 2*raw4 + (b4 - rowsum4): one fused ACT pass straight
                # from PSUM (c4 rounded once to fp32; ~1e-4 logit error, far
                # inside the 2e-2 gate, and nothing thresholds downstream)
                lg = small.tile([C, NBG, NB], F32, tag="lg", bufs=1)
                nc.scalar.activation(lg[:], p4q[:C, :, :],
                                     mybir.ActivationFunctionType.Identity,
                                     bias=c4_sb[:], scale=2.0)
                ptq = pmm.tile([128, NBG, NB], F32, tag="mm", name="ptq")
                for bg in range(NBG):
                    for t in range(NB // 128):
                        nc.tensor.transpose(
                            ptq[:, bg, t * C:(t + 1) * C],
                            lg[:, bg, bass.ts(t, 128)], ident[:C, :C])
                TB = NB // 128  # 4 transpose tiles per bg
                lt = ptq[:, :, :TB * C].rearrange("p g (t c) -> p g t c",
                                                  c=C)
                mxn = small.tile([128, NBG, TB], F32, tag="mxn")
                nc.vector.tensor_reduce(out=mxn[:], in_=lt,
                                        op=mybir.AluOpType.max,
                                        axis=mybir.AxisListType.X,
                                        negate=True)
                sh = small.tile([128, NBG, TB, C], F32, tag="sh")
                nc.vector.tensor_tensor(out=sh[:], in0=lt,
                                        in1=bcast(mxn[:], C),
                                        op=mybir.AluOpType.add)
                ex = small.tile([128, NBG, TB, C], F32, tag="ex")
                nc.scalar.activation(ex[:], sh[:],
                                     mybir.ActivationFunctionType.Exp)
                sm = small.tile([128, NBG, TB], F32, tag="sm")
                nc.vector.tensor_reduce(out=sm[:], in_=ex[:],
                                        op=mybir.AluOpType.add,
                                        axis=mybir.AxisListType.X)
                lsm = small.tile([128, NBG, TB], F32, tag="lsm")
                nc.scalar.activation(lsm[:], sm[:],
                                     mybir.ActivationFunctionType.Ln)
                ot = small.tile([128, NBG, TB, C], F32, tag="ot")
                nc.vector.tensor_tensor(out=ot[:], in0=sh[:],
                                        in1=bcast(lsm[:], C),
                                        op=mybir.AluOpType.subtract)
                for bg in range(NBG):
                    nc.sync.dma_start(
                        out[bg * NB:(bg + 1) * NB, :].rearrange(
                            "(t p) c -> p t c", p=128),
                        ot[:, bg, :, :])
                continue
            # ---- fc4 + log_softmax (per-bg tail) ----
            for bg in range(NBG):
                bsl = bass.ts(bg, NB)
                p4full = pmm.tile([128, NB], F32, tag="mm", name=f"p4_{bg}")
                p4 = p4full[:C, :]
                if fc4_dr:
                    for d in range(DP2):
                        nc.tensor.matmul(p4full[:16, :], w4_sb[:, d, :, :],
                                         s3[:, bg, 2 * d:2 * d + 2, :],
                                         start=(d == 0), stop=(d == DP2 - 1),
                                         perf_mode=mybir.MatmulPerfMode.DoubleRow)
                else:
                    for kc in range(FH):
                        nc.tensor.matmul(p4, w4_sb[:, kc, :], s3[:, bg, kc, :],
                                         start=(kc == 0), stop=(kc == FH - 1))
                # logits = 2*(raw4 - rowsum4/2) + b4: subtract exact (ints),
                # scale+bias rounds once -- bit-identical to (h @ W.T) + b4
                q4 = small.tile([C, NB], F32, tag="q4")
                nc.vector.tensor_scalar(q4[:], p4, rs4h_sb[:], None,
                                        mybir.AluOpType.subtract)
                lg = small.tile([C, NB], F32, tag="lg")
                nc.scalar.activation(lg[:], q4[:],
                                     mybir.ActivationFunctionType.Identity,
                                     bias=b4_sb[:], scale=2.0)
                # transpose 4x [10,128] -> one [128, 4, 10] psum region
                ptf = pmm.tile([128, NB], F32, tag="mm", name=f"pt_{bg}")
                for t in range(NB // 128):
                    nc.tensor.transpose(ptf[:, t * C:(t + 1) * C],
                                        lg[:, bass.ts(t, 128)], ident[:C, :C])
                lt = ptf[:, :4 * C].rearrange("p (t c) -> p t c", c=C)
                # batched log_softmax over the class dim (innermost)
                mxn = small.tile([128, 4], F32, tag="mxn")
                nc.vector.tensor_reduce(out=mxn[:], in_=lt,
                                        op=mybir.AluOpType.max,
                                        axis=mybir.AxisListType.X, negate=True)
                sh = small.tile([128, 4, C], F32, tag="sh")
                nc.vector.tensor_tensor(out=sh[:], in0=lt, in1=bcast(mxn[:], C),
                                        op=mybir.AluOpType.add)
                ex = small.tile([128, 4, C], F32, tag="ex")
                nc.scalar.activation(ex[:], sh[:],
                                     mybir.ActivationFunctionType.Exp)
                sm = small.tile([128, 4], F32, tag="sm")
                nc.vector.tensor_reduce(out=sm[:], in_=ex[:],
                                        op=mybir.AluOpType.add,
                                        axis=mybir.AxisListType.X)
                lsm = small.tile([128, 4], F32, tag="lsm")
                nc.scalar.activation(lsm[:], sm[:],
                                     mybir.ActivationFunctionType.Ln)
                ot = small.tile([128, 4, C], F32, tag="ot")
                nc.vector.tensor_tensor(out=ot[:], in0=sh[:],
                                        in1=bcast(lsm[:], C),
                                        op=mybir.AluOpType.subtract)
                nc.sync.dma_start(
                    out[bg * NB:(bg + 1) * NB, :].rearrange(
                        "(t p) c -> p t c", p=128), ot[:])

    if do_compile:
        nc.compile()
    return nc


def _prep_inputs(inputs, swi=True):
    """Host-side packing: sharding, fp16 split, sign-binarization, fp64
    threshold folding."""
    f64 = np.float64
    x = np.asarray(inputs["x"], np.float32)
    W1s = np.sign(np.asarray(inputs["W1"], np.float32))
    W2s = np.sign(np.asarray(inputs["W2"], np.float32))
    W3s = np.sign(np.asarray(inputs["W3"], np.float32))
    W4s = np.sign(np.asarray(inputs["W4"], np.float32))

    def thr(i):
        g = np.asarray(inputs[f"g{i}"], f64)
        be = np.asarray(inputs[f"be{i}"], f64)
        m = np.asarray(inputs[f"m{i}"], f64)
        v = np.asarray(inputs[f"v{i}"], f64)
        return m - be * np.sqrt(v + BN_EPS) / g

    b1 = np.asarray(inputs["b1"], f64)
    b2 = np.asarray(inputs["b2"], f64)
    b3 = np.asarray(inputs["b3"], f64)
    b4 = np.asarray(inputs["b4"], np.float32)

    # every layer output is {0,1} = (raw >= u); for a {0,1} input layer the
    # true pre-bn activation is h = 2*raw - rowsum + b, so the bn>=0
    # decision h >= t becomes raw >= (rowsum - b + t)/2
    u1 = (thr(1) - b1).astype(np.float32)
    # fc1 output encoding per feature: fg4 groups in ACT1_FG4 emit +-1 via
    # ACT Sign, the rest emit {0,1} via DVE is_ge. For {0,1} rows, double
    # the fc2 weight row (product 2*w*s01 = w*(2*s01)) and shift the
    # threshold by that row-subset's rowsum: h2 = raw2' - rowsum_01 + b2.
    f_idx = np.arange(H) // 128
    s1_is01 = ~np.isin(f_idx // 4, np.asarray(ACT1_FG4))
    W2mod = W2s * np.where(s1_is01[None, :], 2.0, 1.0).astype(np.float32)
    rowsum2_01 = W2s.astype(f64)[:, s1_is01].sum(axis=1)
    u2_64 = rowsum2_01 - b2 + thr(2)
    u2v = u2_64.astype(np.float32)
    rowsum3 = W3s.astype(f64).sum(axis=1)
    u3_64 = (rowsum3 - b3 + thr(3)) / 2.0
    u3 = u3_64.astype(np.float32)

    # ACT-sigmoid threshold path: a feature is eligible iff its threshold is
    # far enough from every integer (matmul outputs are exact ints) that
    # sigmoid(S*(p-u)) provably saturates to {0,1} in fp8, with 4x margin
    # over the fp32 bias-rounding error. Whole 128-feature tiles only.
    def act_mask(u64):
        dz = np.abs(u64 - np.round(u64))
        ok = (dz * ACT_SCALE >= 100.0).reshape(FH, 128).all(axis=1)
        return tuple(f for f in range(FH) if ok[f] and f % 2 == 1)

    _CACHE["act2"] = act_mask(u2_64)
    _CACHE["act3"] = act_mask(u3_64)
    u2s_arr = np.ascontiguousarray(
        (-ACT_SCALE * u2_64).astype(np.float32).reshape(FH, 128).T)
    u3s_arr = np.ascontiguousarray(
        (-ACT_SCALE * u3_64).astype(np.float32).reshape(FH, 128).T)
    # layer 4: logits = 2*(raw4 - rowsum4/2) + b4 = 2*raw4 + c4
    rowsum4 = W4s.astype(f64).sum(axis=1)
    rs4h_v = (rowsum4 / 2.0).astype(np.float32)
    c4_v = (np.asarray(inputs["b4"], f64) - rowsum4).astype(np.float32)

    # fc1 operands: fp16 hi/lo split of x, K = [hi(784) | lo(784) | pad]
    x_hi = x.astype(np.float16)
    x_lo = (x - x_hi.astype(np.float32)).astype(np.float16)
    w1cat = np.zeros((K1P, H), np.float32)
    w1cat[:D_IN] = W1s.T
    w1cat[D_IN:2 * D_IN] = W1s.T
    # [f, p, kc, m]: per-f slice contiguous for long DMA lines
    w1_arr = np.ascontiguousarray(
        w1cat.reshape(KC1, 128, FH, 128).transpose(2, 1, 0, 3)).astype(
        ml_dtypes.float8_e4m3)
    # per-f: move chunk-12 weights into row group f%4 (matches tile_position)
    for f in range(FH):
        j = f % 4
        blk = w1_arr[f, 0:32, KC1 - 1, :].copy()
        w1_arr[f, :, KC1 - 1, :] = 0
        w1_arr[f, 32 * j:32 * j + 32, KC1 - 1, :] = blk

    xcatT = np.zeros((K1P, B), np.float16)
    xcatT[:D_IN] = x_hi.T
    xcatT[D_IN:2 * D_IN] = x_lo.T
    # chunk 12 holds rows 1536:1568 in row-group 0; replicate into groups
    # 1..3 so per-f packed finals can read their own row group
    for i in range(1, 4):
        xcatT[1536 + 32 * i:1536 + 32 * (i + 1)] = xcatT[1536:1568]

    def pack_dr(WsT):
        a = WsT.reshape(DP2, 2, 128, H).transpose(0, 2, 1, 3)
        if not swi:
            return np.ascontiguousarray(a).astype(ml_dtypes.float8_e4m3)
        # SwInterleave: per f-block of 128 cols, pairs (A,B) interleaved
        # with reversed column order: [A127 B127 A126 B126 ... A0 B0]
        a = a.reshape(DP2, 128, 2, FH, 128)          # [d, p, j, f, m]
        a = a[:, :, :, :, ::-1]                      # reverse m
        a = a.transpose(0, 1, 3, 4, 2)               # [d, p, f, m, j]
        return np.ascontiguousarray(
            a.reshape(DP2, 128, FH, 256)).astype(ml_dtypes.float8_e4m3)

    w2_arr = pack_dr(W2mod.T)
    w3_arr = pack_dr(W3s.T)
    w4p = np.zeros((H, 16), np.float32)
    w4p[:, :C] = W4s.T
    w4_arr = np.ascontiguousarray(
        w4p.reshape(DP2, 2, 128, 16).transpose(0, 2, 1, 3)).astype(
        ml_dtypes.float8_e4m3)

    u1_arr = np.ascontiguousarray(u1.reshape(FH, 128).T)
    u1n_arr = np.ascontiguousarray((-u1).reshape(FH, 128).T)
    u2_arr = np.ascontiguousarray(u2v.reshape(FH, 128).T)
    u3_arr = np.ascontiguousarray(u3.reshape(FH, 128).T)
    rs4h_arr = np.ascontiguousarray(rs4h_v.reshape(C, 1))
    b4_arr = np.ascontiguousarray(b4.reshape(C, 1))
    c4_arr = np.ascontiguousarray(c4_v.reshape(C, 1))

    shared = {"w1": w1_arr, "w2": w2_arr, "w3": w3_arr, "w4": w4_arr,
              "u1": u1_arr, "u2": u2_arr, "u3": u3_arr, "rs4h": rs4h_arr,
              "b4": b4_arr, "c4": c4_arr, "u2s": u2s_arr, "u3s": u3s_arr,
              "u1n": u1n_arr}
    in_maps = []
    for c in range(N_CORES):
        # [bg, p, kc, b]: per-bg slice contiguous
        xc = np.ascontiguousarray(
            np.ascontiguousarray(xcatT[:, c * BS:(c + 1) * BS])
            .reshape(KC1, 128, NBG, NB).transpose(2, 1, 0, 3))
        in_maps.append({"xcat": xc, **shared})
    return in_maps


def kernel(**inputs):
    in_maps = _prep_inputs(inputs)
    if "nc" not in _CACHE:
        _CACHE["nc"] = _build_program()
    nc = _CACHE["nc"]
    res = run_bass_kernel_spmd(nc, in_maps, list(range(N_CORES)))
    return np.concatenate([res.results[c]["out"] for c in range(N_CORES)],
                          axis=0).astype(np.float32)

